# revision 9
# baseline (speedup 1.0000x reference)
"""Conv2d(256->256, 3x3, pad=1) on 8 TRN2 NeuronCores — Winograd F(2,3) along W.

Sharding: data-parallel over output rows (H), 28 rows/core, weights replicated
(kept from the direct-conv baseline: it keeps M=128 output channels per matmul).

Algorithm: 1D Winograd F(2,3) on the W axis, direct convolution on H (3 taps)
and channels. Per output pair the direct conv needs 9 taps x 2 c-blocks = 18
accumulation columns; Winograd needs 4 points/2 outputs x 3 kh x 2 cb = 12 —
a 1.5x reduction in PE work (504 -> 336 matmuls of N=448, ~189 ns cadence).

  z0 = d0 - d2, z1 = d1 + d2, z2 = d2 - d1, z3 = d1 - d3   (input transform)
  Gg = [g0, (g0+g1+g2)/2, (g0-g1+g2)/2, g2]                 (weights, host)
  m_j[o,h,t] = sum_{c,kh} z_j[c,h+kh,t] * Gg_j[o,c,kh]      (PE, PSUM fp32)
  out[2t]   = m0 + m1 + m2                                  (output transform)
  out[2t+1] = m1 - m2 - m3

Everything is bf16 except PSUM accumulation: bf16 lets LDWEIGHTS pipeline
ahead of matmuls (f32r self-loads, ~205ns/MM vs 189), and DVE tensor_tensor
only reaches 2x_1P speed when every operand is 2-byte, stride-1, SBUF. To keep
every DVE access packed, x arrives parity-split (even/odd W columns in
separate planes, so d0..d3 are stride-1 views) and the output leaves
parity-split the same way (host re-interleaves + converts to fp32).

Engine split (PE ~63.5us is the roofline; others hide under it):
  DVE : input transform (2x bf16), final even/odd combines (2x bf16)
  ACT : PSUM->SBUF bf16 staging of m0..m3 (DVE can't read 2 PSUM operands,
        and PSUM operands drop DVE to 1x anyway)
  Sync: all DMA issue (inputs first, then per-group output stores)
  Pool: nothing — GpSimd tensor ops lock the SBUF port pair DVE needs and
        double everyone's latency (measured), so it stays idle.
Warmup matmuls on a memset tile keep the HAM clock-gate at 8/8 through the
~3us head DMA gate so the real stream never drops to K=4/8.
"""

import sys

sys.path.insert(0, "/opt/trn_rl_repo")

import numpy as np
import ml_dtypes

import concourse.mybir as mybir
from concourse import bacc
from concourse.tile import TileContext
from concourse.bass_utils import run_bass_kernel_spmd

N_CORES = 8
C, H, W = 256, 224, 224
O = 256
KH = 3
HS = H // N_CORES          # 28 output rows per core
HR = 4                     # output rows per PSUM tile group
HG = HS // HR              # 7 h-groups
T = W // 2                 # 112 Winograd tiles per row
J = 4                      # Winograd points per tile
CB = C // 128
OB = O // 128
HP = HS + 2                # x rows per core (1 halo each side)
TP = T + 1                 # parity-plane width (113 even / 113 odd columns)

_CACHE = {}
LAST_RESULTS = None
TRACE = False

BF16 = mybir.dt.bfloat16
F32 = mybir.dt.float32


def _build():
    nc = bacc.Bacc(None, target_bir_lowering=False)

    # x parity-split: xs[cb, c, r, p, t] = xpad[cb*128+c, r, 2t+p]
    xs = nc.dram_tensor("xs", [CB, 128, HP, 2, TP], BF16, kind="ExternalInput")
    # w[cb, ob, c, j*3+kh, o]
    w = nc.dram_tensor("w", [CB, OB, 128, J * KH, 128], BF16, kind="ExternalInput")
    # out parity-split: out[ob, o, h, p, t] = y[ob*128+o, h, 2t+p]
    out = nc.dram_tensor("out", [OB, 128, HS, 2, T], BF16, kind="ExternalOutput")

    n_warm = 7
    add = mybir.AluOpType.add
    sub = mybir.AluOpType.subtract

    with TileContext(nc) as tc:
        with (
            tc.tile_pool(name="warm", bufs=1) as pwarm,
            tc.tile_pool(name="win", bufs=1) as pw,
            tc.tile_pool(name="xin", bufs=1) as px,
            tc.tile_pool(name="zbuf", bufs=1) as pz,
            tc.tile_pool(name="psumw", bufs=1, space="PSUM") as ppw,
            tc.tile_pool(name="psum", bufs=7, space="PSUM") as pp,
            tc.tile_pool(name="tmp", bufs=4) as pt,
            tc.tile_pool(name="outp", bufs=4) as po,
        ):
            # PE warmup against the HAM clock-gate while input DMAs stream.
            wt0 = pwarm.tile([128, 448], BF16, tag="warm")
            ps0 = ppw.tile([128, 448], F32, tag="warmps")
            nc.gpsimd.memset(wt0[:], 0.0)
            for _ in range(n_warm):
                nc.tensor.matmul(ps0[:], wt0[:, :128], wt0[:], start=True, stop=True)

            x_sb = [
                px.tile([128, HP, 2, TP], BF16, tag=f"x{b}", name=f"x{b}")
                for b in range(CB)
            ]
            # One tile per (cb, ob) so each weight DMA lands fully
            # contiguous (a strided dst explodes descriptor count/latency).
            w_sb = [
                [
                    pw.tile(
                        [128, J * KH, 128], BF16, tag=f"w{b}o{ob}", name=f"w{b}o{ob}"
                    )
                    for ob in range(OB)
                ]
                for b in range(CB)
            ]
            z_sb = [
                pz.tile([128, J, HP, T], BF16, tag=f"z{b}", name=f"z{b}")
                for b in range(CB)
            ]

            def dma_w(b, ob, j0, j1, eng=None):
                (eng or nc.sync).dma_start(
                    out=w_sb[b][ob][:, j0 * KH : j1 * KH, :],
                    in_=w[b, ob, :, j0 * KH : j1 * KH, :],
                )

            def dma_x(b, r0, r1, eng=None):
                (eng or nc.sync).dma_start(
                    out=x_sb[b][:, r0:r1, :, :], in_=xs[b, :, r0:r1, :, :]
                )

            # Head, ordered by first consumption: first x rows + j0/j1
            # weights for cb0, then cb1, then the rest; 8-row steady chunks.
            # GpSimd's queue exits the framework preamble ~1.5-3us before
            # Sync's, so the critical head transfers issue from there (SWDGE).
            dma_x(0, 0, 6, nc.gpsimd)
            dma_x(1, 0, 6, nc.gpsimd)
            dma_w(0, 0, 0, 2, nc.gpsimd)
            dma_w(1, 0, 0, 2, nc.gpsimd)
            dma_w(0, 0, 2, 4, nc.gpsimd)
            dma_w(1, 0, 2, 4, nc.gpsimd)
            dma_w(0, 1, 0, 4, nc.gpsimd)
            dma_w(1, 1, 0, 4, nc.gpsimd)
            for r in range(6, HP, 8):
                r1 = min(r + 8, HP)
                dma_x(0, r, r1)
                dma_x(1, r, r1)

            # Input transform, all stride-1 bf16 (2x_1P):
            #   d0 = even[t], d1 = odd[t], d2 = even[t+1], d3 = odd[t+1]
            #   z0 = d0-d2, z1 = d1+d2, z2 = d2-d1, z3 = d1-d3
            def ztrans(b, r0, r1):
                x_ = x_sb[b]
                z_ = z_sb[b]
                e0 = x_[:, r0:r1, 0, 0:T]
                e1 = x_[:, r0:r1, 0, 1 : T + 1]
                o0 = x_[:, r0:r1, 1, 0:T]
                o1 = x_[:, r0:r1, 1, 1 : T + 1]
                nc.vector.tensor_tensor(z_[:, 0, r0:r1, :], e0, e1, sub)
                nc.vector.tensor_tensor(z_[:, 1, r0:r1, :], o0, e1, add)
                nc.vector.tensor_tensor(z_[:, 2, r0:r1, :], e1, o0, sub)
                nc.vector.tensor_tensor(z_[:, 3, r0:r1, :], o0, o1, sub)

            ztrans(0, 0, 6)
            ztrans(1, 0, 6)

            def mm_group(ps_j, h0, ob):
                # 4 psum tiles (one per Winograd point), 6 accumulating
                # matmuls each (cb-major so the head can start on cb0).
                for j in range(J):
                    idx = 0
                    for b in range(CB):
                        for kh in range(KH):
                            nc.tensor.matmul(
                                ps_j[j][:],
                                w_sb[b][ob][:, j * KH + kh, :],
                                z_sb[b][:, j, h0 + kh : h0 + kh + HR, :],
                                start=(idx == 0),
                                stop=(idx == CB * KH - 1),
                            )
                            idx += 1

            def finish_group(ps_j, h0, ob):
                # ACT downcasts m0..m3 to bf16 SBUF; DVE combines at 2x and
                # writes parity-split planes (even=ot[...,0,:], odd=[...,1,:]).
                ot = po.tile([128, HR, 2, T], BF16, tag="ot", name="ot")
                s = [
                    pt.tile([128, HR, T], BF16, tag=f"s{k}", name=f"s{k}")
                    for k in range(4)
                ]
                for k in range(4):
                    nc.scalar.copy(s[k][:], ps_j[k][:])
                t0 = pt.tile([128, HR, T], BF16, tag="t0", name="t0")
                t1 = pt.tile([128, HR, T], BF16, tag="t1", name="t1")
                nc.vector.tensor_tensor(t0[:], s[0][:], s[1][:], add)
                nc.vector.tensor_tensor(ot[:, :, 0, :], t0[:], s[2][:], add)
                nc.vector.tensor_tensor(t1[:], s[1][:], s[2][:], sub)
                nc.vector.tensor_tensor(ot[:, :, 1, :], t1[:], s[3][:], sub)
                nc.sync.dma_start(out=out[ob, :, h0 : h0 + HR, :, :], in_=ot[:])

            for hg in range(HG):
                h0 = hg * HR
                for ob in range(OB):
                    ps_j = [
                        pp.tile([128, HR, T], F32, tag="ps", name="ps")
                        for _ in range(J)
                    ]
                    mm_group(ps_j, h0, ob)
                    if ob == 0 and hg % 2 == 0 and hg + 1 < HG:
                        # Upcoming z rows (8-row chunks), emitted ahead of
                        # this group's output transforms so DVE never gates
                        # the PE.
                        r0 = 8 * (hg // 2) + 6
                        r1 = min(r0 + 8, HP)
                        ztrans(0, r0, r1)
                        ztrans(1, r0, r1)
                    finish_group(ps_j, h0, ob)

    nc.compile()
    return nc


def kernel(x: np.ndarray, kernel: np.ndarray) -> np.ndarray:
    global LAST_RESULTS
    if "nc" not in _CACHE:
        _CACHE["nc"] = _build()
    nc = _CACHE["nc"]

    x = np.ascontiguousarray(x, dtype=np.float32)
    kw_arr = np.ascontiguousarray(kernel, dtype=np.float32)

    xp = np.pad(x, ((0, 0), (1, 1), (1, 1)))          # [C, H+2, W+2]

    # Winograd weight transform along kw: Gg[o,c,kh,j]
    g = kw_arr  # [O, C, 3, 3]
    gg = np.empty((O, C, KH, J), dtype=np.float32)
    gg[..., 0] = g[..., 0]
    gg[..., 1] = 0.5 * (g[..., 0] + g[..., 1] + g[..., 2])
    gg[..., 2] = 0.5 * (g[..., 0] - g[..., 1] + g[..., 2])
    gg[..., 3] = g[..., 2]
    # w_t[cb, ob, c, j*3+kh, o]
    w_t = np.ascontiguousarray(
        gg.reshape(OB, 128, CB, 128, KH, J)
        .transpose(2, 0, 3, 5, 4, 1)
        .reshape(CB, OB, 128, J * KH, 128)
        .astype(ml_dtypes.bfloat16)
    )

    in_maps = []
    for i in range(N_CORES):
        sl = xp[:, i * HS : i * HS + HP, :]           # [C, HP, 226]
        xe = np.empty((C, HP, 2, TP), dtype=np.float32)
        xe[:, :, 0, :] = sl[:, :, 0::2]               # even cols 0,2,...,224
        xe[:, :, 1, :] = sl[:, :, 1::2]               # odd  cols 1,3,...,225
        xs_i = np.ascontiguousarray(
            xe.reshape(CB, 128, HP, 2, TP).astype(ml_dtypes.bfloat16)
        )
        in_maps.append({"xs": xs_i, "w": w_t})

    # The axon-tunneled device occasionally wedges with a transient
    # NRT_EXEC_UNIT_UNRECOVERABLE; a retry on a fresh execute recovers it.
    last_err = None
    for _ in range(3):
        try:
            results = run_bass_kernel_spmd(
                nc, in_maps, core_ids=list(range(N_CORES)), trace=TRACE
            )
            break
        except Exception as e:  # noqa: BLE001
            last_err = e
    else:
        raise last_err
    LAST_RESULTS = results

    parts = []
    for r in results.results:
        y = r["out"].astype(np.float32).reshape(O, HS, 2, T)
        yi = np.empty((O, HS, W), dtype=np.float32)
        yi[:, :, 0::2] = y[:, :, 0, :]
        yi[:, :, 1::2] = y[:, :, 1, :]
        parts.append(yi)
    return np.concatenate(parts, axis=1)


# revision 10
# speedup vs baseline: 1.2422x; 1.2422x over previous
"""Conv2d(256->256, 3x3, pad=1) on 8 TRN2 NeuronCores — Winograd F(2,3) along W.

Sharding: data-parallel over output rows (H), 28 rows/core, weights replicated
(kept from the direct-conv baseline: it keeps M=128 output channels per matmul).

Algorithm: 1D Winograd F(2,3) on the W axis, direct convolution on H (3 taps)
and channels. Per output pair the direct conv needs 9 taps x 2 c-blocks = 18
accumulation columns; Winograd needs 4 points/2 outputs x 3 kh x 2 cb = 12 —
a 1.5x reduction in PE work (504 -> 336 matmuls of N=448, ~189 ns cadence).

  z0 = d0 - d2, z1 = d1 + d2, z2 = d2 - d1, z3 = d1 - d3   (input transform)
  Gg = [g0, (g0+g1+g2)/2, (g0-g1+g2)/2, g2]                 (weights, host)
  m_j[o,h,t] = sum_{c,kh} z_j[c,h+kh,t] * Gg_j[o,c,kh]      (PE, PSUM fp32)
  out[2t]   = m0 + m1 + m2                                  (output transform)
  out[2t+1] = m1 - m2 - m3

Everything is bf16 except PSUM accumulation: bf16 lets LDWEIGHTS pipeline
ahead of matmuls (f32r self-loads, ~205ns/MM vs 189), and DVE tensor_tensor
only reaches 2x_1P speed when every operand is 2-byte, stride-1, SBUF. To keep
every DVE access packed, x arrives parity-split (even/odd W columns in
separate planes, so d0..d3 are stride-1 views) and the output leaves
parity-split the same way (host re-interleaves + converts to fp32).

Engine split (PE ~63.5us is the roofline; others hide under it):
  DVE : input transform (2x bf16), final even/odd combines (2x bf16)
  ACT : PSUM->SBUF bf16 staging of m0..m3 (DVE can't read 2 PSUM operands,
        and PSUM operands drop DVE to 1x anyway)
  Sync: all DMA issue (inputs first, then per-group output stores)
  Pool: nothing — GpSimd tensor ops lock the SBUF port pair DVE needs and
        double everyone's latency (measured), so it stays idle.
Warmup matmuls on a memset tile keep the HAM clock-gate at 8/8 through the
~3us head DMA gate so the real stream never drops to K=4/8.
"""

import sys

sys.path.insert(0, "/opt/trn_rl_repo")

import numpy as np
import ml_dtypes

import concourse.mybir as mybir
from concourse import bacc
from concourse.tile import TileContext
from concourse.bass_utils import run_bass_kernel_spmd

N_CORES = 8
C, H, W = 256, 224, 224
O = 256
KH = 3
HS = H // N_CORES          # 28 output rows per core
HR = 4                     # output rows per PSUM tile group
HG = HS // HR              # 7 h-groups
T = W // 2                 # 112 Winograd tiles per row
J = 4                      # Winograd points per tile
CB = C // 128
OB = O // 128
HP = HS + 2                # x rows per core (1 halo each side)
TP = T + 1                 # parity-plane width (113 even / 113 odd columns)

_CACHE = {}
LAST_RESULTS = None
TRACE = False

BF16 = mybir.dt.bfloat16
F32 = mybir.dt.float32


def _build():
    nc = bacc.Bacc(None, target_bir_lowering=False)

    # x parity-split: xs[cb, c, r, p, t] = xpad[cb*128+c, r, 2t+p]
    xs = nc.dram_tensor("xs", [CB, 128, HP, 2, TP], BF16, kind="ExternalInput")
    # w[cb, ob, c, j*3+kh, o]
    w = nc.dram_tensor("w", [CB, OB, 128, J * KH, 128], BF16, kind="ExternalInput")
    # out parity-split: out[ob, o, h, p, t] = y[ob*128+o, h, 2t+p]
    out = nc.dram_tensor("out", [OB, 128, HS, 2, T], BF16, kind="ExternalOutput")

    n_warm = 7
    add = mybir.AluOpType.add
    sub = mybir.AluOpType.subtract

    with TileContext(nc) as tc:
        with (
            tc.tile_pool(name="warm", bufs=1) as pwarm,
            tc.tile_pool(name="win", bufs=1) as pw,
            tc.tile_pool(name="xin", bufs=1) as px,
            tc.tile_pool(name="zbuf", bufs=1) as pz,
            tc.tile_pool(name="psumw", bufs=1, space="PSUM") as ppw,
            tc.tile_pool(name="psum", bufs=7, space="PSUM") as pp,
            tc.tile_pool(name="tmp", bufs=4) as pt,
            tc.tile_pool(name="outp", bufs=4) as po,
        ):
            # PE warmup against the HAM clock-gate while input DMAs stream.
            wt0 = pwarm.tile([128, 448], BF16, tag="warm")
            ps0 = ppw.tile([128, 448], F32, tag="warmps")
            nc.vector.memset(wt0[:], 0.0)
            for _ in range(n_warm):
                nc.tensor.matmul(ps0[:], wt0[:, :128], wt0[:], start=True, stop=True)

            x_sb = [
                px.tile([128, HP, 2, TP], BF16, tag=f"x{b}", name=f"x{b}")
                for b in range(CB)
            ]
            # One tile per (cb, ob) so each weight DMA lands fully
            # contiguous (a strided dst explodes descriptor count/latency).
            w_sb = [
                [
                    pw.tile(
                        [128, J * KH, 128], BF16, tag=f"w{b}o{ob}", name=f"w{b}o{ob}"
                    )
                    for ob in range(OB)
                ]
                for b in range(CB)
            ]
            z_sb = [
                pz.tile([128, J, HP, T], BF16, tag=f"z{b}", name=f"z{b}")
                for b in range(CB)
            ]

            def dma_w(b, ob, j0, j1, eng=None):
                (eng or nc.sync).dma_start(
                    out=w_sb[b][ob][:, j0 * KH : j1 * KH, :],
                    in_=w[b, ob, :, j0 * KH : j1 * KH, :],
                )

            def dma_x(b, r0, r1, eng=None):
                (eng or nc.sync).dma_start(
                    out=x_sb[b][:, r0:r1, :, :], in_=xs[b, :, r0:r1, :, :]
                )

            # Head, ordered by first consumption: first x rows + j0/j1
            # weights for cb0, then cb1, then the rest; 8-row steady chunks.
            dma_x(0, 0, 6)
            dma_x(1, 0, 6)
            dma_w(0, 0, 0, 2)
            dma_w(1, 0, 0, 2)
            dma_w(0, 0, 2, 4)
            dma_w(1, 0, 2, 4)
            dma_w(0, 1, 0, 4)
            dma_w(1, 1, 0, 4)
            for r in range(6, HP, 8):
                r1 = min(r + 8, HP)
                dma_x(0, r, r1)
                dma_x(1, r, r1)

            # Input transform, all stride-1 bf16 (2x_1P):
            #   d0 = even[t], d1 = odd[t], d2 = even[t+1], d3 = odd[t+1]
            #   z0 = d0-d2, z1 = d1+d2, z2 = d2-d1, z3 = d1-d3
            def ztrans(b, r0, r1):
                x_ = x_sb[b]
                z_ = z_sb[b]
                e0 = x_[:, r0:r1, 0, 0:T]
                e1 = x_[:, r0:r1, 0, 1 : T + 1]
                o0 = x_[:, r0:r1, 1, 0:T]
                o1 = x_[:, r0:r1, 1, 1 : T + 1]
                nc.vector.tensor_tensor(z_[:, 0, r0:r1, :], e0, e1, sub)
                nc.vector.tensor_tensor(z_[:, 1, r0:r1, :], o0, e1, add)
                nc.vector.tensor_tensor(z_[:, 2, r0:r1, :], e1, o0, sub)
                nc.vector.tensor_tensor(z_[:, 3, r0:r1, :], o0, o1, sub)

            ztrans(0, 0, 6)
            ztrans(1, 0, 6)

            def mm_group(ps_j, h0, ob):
                # 4 psum tiles (one per Winograd point), 6 accumulating
                # matmuls each (cb-major so the head can start on cb0).
                for j in range(J):
                    idx = 0
                    for b in range(CB):
                        for kh in range(KH):
                            nc.tensor.matmul(
                                ps_j[j][:],
                                w_sb[b][ob][:, j * KH + kh, :],
                                z_sb[b][:, j, h0 + kh : h0 + kh + HR, :],
                                start=(idx == 0),
                                stop=(idx == CB * KH - 1),
                            )
                            idx += 1

            def finish_group(ps_j, h0, ob):
                # ACT downcasts m0..m3 to bf16 SBUF; DVE combines at 2x and
                # writes parity-split planes (even=ot[...,0,:], odd=[...,1,:]).
                ot = po.tile([128, HR, 2, T], BF16, tag="ot", name="ot")
                s = [
                    pt.tile([128, HR, T], BF16, tag=f"s{k}", name=f"s{k}")
                    for k in range(4)
                ]
                for k in range(4):
                    nc.scalar.copy(s[k][:], ps_j[k][:])
                t0 = pt.tile([128, HR, T], BF16, tag="t0", name="t0")
                t1 = pt.tile([128, HR, T], BF16, tag="t1", name="t1")
                nc.vector.tensor_tensor(t0[:], s[0][:], s[1][:], add)
                nc.vector.tensor_tensor(ot[:, :, 0, :], t0[:], s[2][:], add)
                nc.vector.tensor_tensor(t1[:], s[1][:], s[2][:], sub)
                nc.vector.tensor_tensor(ot[:, :, 1, :], t1[:], s[3][:], sub)
                nc.sync.dma_start(out=out[ob, :, h0 : h0 + HR, :, :], in_=ot[:])

            for hg in range(HG):
                h0 = hg * HR
                for ob in range(OB):
                    ps_j = [
                        pp.tile([128, HR, T], F32, tag="ps", name="ps")
                        for _ in range(J)
                    ]
                    mm_group(ps_j, h0, ob)
                    if ob == 0 and hg % 2 == 0 and hg + 1 < HG:
                        # Upcoming z rows (8-row chunks), emitted ahead of
                        # this group's output transforms so DVE never gates
                        # the PE.
                        r0 = 8 * (hg // 2) + 6
                        r1 = min(r0 + 8, HP)
                        ztrans(0, r0, r1)
                        ztrans(1, r0, r1)
                    finish_group(ps_j, h0, ob)

    nc.compile()
    return nc


def kernel(x: np.ndarray, kernel: np.ndarray) -> np.ndarray:
    global LAST_RESULTS
    if "nc" not in _CACHE:
        _CACHE["nc"] = _build()
    nc = _CACHE["nc"]

    x = np.ascontiguousarray(x, dtype=np.float32)
    kw_arr = np.ascontiguousarray(kernel, dtype=np.float32)

    xp = np.pad(x, ((0, 0), (1, 1), (1, 1)))          # [C, H+2, W+2]

    # Winograd weight transform along kw: Gg[o,c,kh,j]
    g = kw_arr  # [O, C, 3, 3]
    gg = np.empty((O, C, KH, J), dtype=np.float32)
    gg[..., 0] = g[..., 0]
    gg[..., 1] = 0.5 * (g[..., 0] + g[..., 1] + g[..., 2])
    gg[..., 2] = 0.5 * (g[..., 0] - g[..., 1] + g[..., 2])
    gg[..., 3] = g[..., 2]
    # w_t[cb, ob, c, j*3+kh, o]
    w_t = np.ascontiguousarray(
        gg.reshape(OB, 128, CB, 128, KH, J)
        .transpose(2, 0, 3, 5, 4, 1)
        .reshape(CB, OB, 128, J * KH, 128)
        .astype(ml_dtypes.bfloat16)
    )

    in_maps = []
    for i in range(N_CORES):
        sl = xp[:, i * HS : i * HS + HP, :]           # [C, HP, 226]
        xe = np.empty((C, HP, 2, TP), dtype=np.float32)
        xe[:, :, 0, :] = sl[:, :, 0::2]               # even cols 0,2,...,224
        xe[:, :, 1, :] = sl[:, :, 1::2]               # odd  cols 1,3,...,225
        xs_i = np.ascontiguousarray(
            xe.reshape(CB, 128, HP, 2, TP).astype(ml_dtypes.bfloat16)
        )
        in_maps.append({"xs": xs_i, "w": w_t})

    # The axon-tunneled device occasionally wedges with a transient
    # NRT_EXEC_UNIT_UNRECOVERABLE; a retry on a fresh execute recovers it.
    last_err = None
    for _ in range(3):
        try:
            results = run_bass_kernel_spmd(
                nc, in_maps, core_ids=list(range(N_CORES)), trace=TRACE
            )
            break
        except Exception as e:  # noqa: BLE001
            last_err = e
    else:
        raise last_err
    LAST_RESULTS = results

    parts = []
    for r in results.results:
        y = r["out"].astype(np.float32).reshape(O, HS, 2, T)
        yi = np.empty((O, HS, W), dtype=np.float32)
        yi[:, :, 0::2] = y[:, :, 0, :]
        yi[:, :, 1::2] = y[:, :, 1, :]
        parts.append(yi)
    return np.concatenate(parts, axis=1)
